# revision 1
# baseline (speedup 1.0000x reference)
"""CorrelationHead Trainium2 kernel.

Math: SpatialCorrelationSampler(patch=16, dil=2) on 7x7 maps zero-pads x2 by
(14,16). The displaced sample x2pad[i+2ph, j+2pw] is nonzero only when it
lands in the true 7x7 patch, so corr[b] (12544 features) has exactly 2401
distinct nonzero values = Gram matrix G[b][ij,kl] = sum_c x1[b,c,ij]*x2[b,c,kl]
(valid only when parity matches; invalid ones never appear in corr).
fc1(corr) therefore equals sum_{ij,kl} G[b][ij,kl] * W1eff[kl,ij,rep] with
W1eff[kl,ij,:] = W1[:, ((ph*16+pw)*49+ij)] for parity-valid (ij,kl), else 0.

Sharding: pure data-parallel over the 1024 RoIs -> 128 per each of 8 cores.
Weights replicated. Each core: per-RoI Gram matmuls on PE (K=256 contraction,
2 accumulating matmuls), evict to SBUF, then fc1 as 49 accumulated K=49
matmuls, bias via ones-row matmul, ReLU on ACT, PE-transpose, fc2, ReLU,
transpose, fc3.
"""

import os
import numpy as np

import concourse.bass as bass
import concourse.mybir as mybir
from concourse.bass_utils import run_bass_kernel_spmd

# ---------------------------------------------------------------- constants
P = 16
DIL = 2
H = 7
C = 256
B = 1024
REP = 1024
HW = H * H  # 49
N_CORES = 8
BL = B // N_CORES  # 128 RoIs per core

CH = 16                 # RoIs per input chunk
NCHUNK = BL // CH       # 8
NG = 4                  # RoIs per PSUM gram group
NGROUP = BL // NG       # 32
GPC = CH // NG          # groups per chunk = 4
PSG_RING = 3

SL = 8                  # fc1 ij's per streamed W1 slice
SLICES = [8, 8, 8, 8, 8, 8, 1]   # 49 ijs
NSLICE = len(SLICES)

F32 = mybir.dt.float32
BF16 = mybir.dt.bfloat16

# dtype of matmul operands on device ("f32" or "bf16")
DT_MODE = os.environ.get("CORR_DT", "bf16")
EVICT_MODE = os.environ.get("CORR_EVICT", "perb")  # "perb" | "batch" | "split"
W1_ENG = os.environ.get("CORR_W1ENG", "act")       # "act" | "gpsimd" | "sp"

LAST_EXEC_NS = None
_CACHE = {}


# ---------------------------------------------------------------- host prep
def _w1eff(W1, np_dt):
    """[49 kl, 49 ij, 1024] with zeros for parity-invalid (ij,kl)."""
    w = np.zeros((HW, HW, REP), dtype=np_dt)
    for i in range(H):
        for j in range(H):
            ij = i * H + j
            for k in range(H):
                if (k - i) % 2:
                    continue
                ph = (k - i) // 2 + 7
                for l in range(H):
                    if (l - j) % 2:
                        continue
                    pw = (l - j) // 2 + 7
                    kl = k * H + l
                    f = (ph * P + pw) * HW + ij
                    w[kl, ij, :] = W1[:, f]
    return w


# ---------------------------------------------------------------- device IR
def _build(dt, debug=False):
    nc = bass.Bass()

    x1h = nc.dram_tensor("x1h", [2, 128, BL * HW], dt, kind="ExternalInput")
    x2h = nc.dram_tensor("x2h", [2, 128, BL * HW], dt, kind="ExternalInput")
    w1h = nc.dram_tensor("w1h", [HW, HW * REP], dt, kind="ExternalInput")
    w2h = nc.dram_tensor("w2h", [128, 8 * REP], dt, kind="ExternalInput")
    w3h = nc.dram_tensor("w3h", [128, 8 * 4], dt, kind="ExternalInput")
    b1h = nc.dram_tensor("b1h", [1, REP], dt, kind="ExternalInput")
    b2h = nc.dram_tensor("b2h", [1, REP], dt, kind="ExternalInput")
    b3h = nc.dram_tensor("b3h", [1, 4], dt, kind="ExternalInput")
    onesh = nc.dram_tensor("onesh", [1, 128], dt, kind="ExternalInput")
    identh = nc.dram_tensor("identh", [128, 128], dt, kind="ExternalInput")
    zbh = nc.dram_tensor("zbh", [128, 1], F32, kind="ExternalInput")
    outh = nc.dram_tensor("outh", [128, 4], F32, kind="ExternalOutput")
    if debug:
        gsadbg = nc.dram_tensor("gsadbg", [HW, HW, 128], dt, kind="ExternalOutput")
        relu1dbg = nc.dram_tensor("relu1dbg", [128, REP], dt, kind="ExternalOutput")
        r1Tdbg = nc.dram_tensor("r1Tdbg", [128, REP], dt, kind="ExternalOutput")
        relu2dbg = nc.dram_tensor("relu2dbg", [128, REP], dt, kind="ExternalOutput")

    CW = CH * HW  # 784 columns per chunk

    from contextlib import ExitStack

    with ExitStack() as ctx:
        sb = lambda name, shape, d: ctx.enter_context(nc.sbuf_tensor(name, shape, d))
        ps = lambda name, shape, d: ctx.enter_context(nc.psum_tensor(name, shape, d))
        sem = lambda name: ctx.enter_context(nc.semaphore(name))

        x1s0 = sb("x1s0", [128, 2, CW], dt)
        x1s1 = sb("x1s1", [128, 2, CW], dt)
        x2s0 = sb("x2s0", [128, 2, CW], dt)
        x2s1 = sb("x2s1", [128, 2, CW], dt)
        gsa = sb("gsa", [HW, HW, 128], dt)
        w1_resident = dt != F32
        if w1_resident:
            w1r = sb("w1r", [HW, HW * REP], dt)
        else:
            w1s0 = sb("w1s0", [HW, SL * REP], dt)
            w1s1 = sb("w1s1", [HW, SL * REP], dt)
        w2s = sb("w2s", [128, 8 * REP], dt)
        w3s = sb("w3s", [128, 8 * 4], dt)
        b1s = sb("b1s", [1, REP], dt)
        b2s = sb("b2s", [1, REP], dt)
        b3s = sb("b3s", [1, 4], dt)
        ones = sb("ones", [1, 128], dt)
        idents = sb("idents", [128, 128], dt)
        zbias = sb("zbias", [128, 1], F32)
        relu1 = sb("relu1", [128, REP], dt)
        r1T = sb("r1T", [128, REP], dt)
        relu2 = sb("relu2", [128, REP], dt)
        r2T = sb("r2T", [128, REP], dt)
        outs = sb("outs", [128, 4], F32)
        psG0 = ps("psG0", [HW, NG, HW], F32)
        psG1 = ps("psG1", [HW, NG, HW], F32)
        psG2 = ps("psG2", [HW, NG, HW], F32)
        psF0 = ps("psF0", [128, 512], F32)
        psF1 = ps("psF1", [128, 512], F32)
        psT0 = ps("psT0", [128, 128], dt)
        psT1 = ps("psT1", [128, 128], dt)
        psO = ps("psO", [128, 4], F32)
        s_x0 = sem("s_x0")
        s_x1 = sem("s_x1")
        s_xd = sem("s_xd")
        s_w = sem("s_w")
        s_w1a = sem("s_w1a")
        s_w1b = sem("s_w1b")
        s_w1u = sem("s_w1u")
        s_g = sem("s_g")
        s_e = sem("s_e")
        s_ed = sem("s_ed")
        s_f1 = sem("s_f1")
        s_r1 = sem("s_r1")
        s_t1 = sem("s_t1")
        s_c1 = sem("s_c1")
        s_f2 = sem("s_f2")
        s_r2 = sem("s_r2")
        s_t2 = sem("s_t2")
        s_c2 = sem("s_c2")
        s_f3 = sem("s_f3")
        s_oe = sem("s_oe")
        s_o = sem("s_o")
        block = ctx.enter_context(nc.Block())
        x1s = [x1s0, x1s1]
        x2s = [x2s0, x2s1]
        w1s = None if w1_resident else [w1s0, w1s1]
        sxs = [s_x0, s_x1]
        sw1s = [s_w1a, s_w1b]
        psG = [psG0, psG1, psG2]
        psF = [psF0, psF1]
        psT = [psT0, psT1]
        N_WDMA = 8 * 16  # 8 initial weight DMAs

        # ---------------- SP: input + weight DMAs, final output
        @block.sync
        def _(sp):
            sp.dma_start(w2s[:, :], w2h[:, :]).then_inc(s_w, 16)
            sp.dma_start(w3s[:, :], w3h[:, :]).then_inc(s_w, 16)
            sp.dma_start(b1s[:, :], b1h[:, :]).then_inc(s_w, 16)
            sp.dma_start(b2s[:, :], b2h[:, :]).then_inc(s_w, 16)
            sp.dma_start(b3s[:, :], b3h[:, :]).then_inc(s_w, 16)
            sp.dma_start(ones[:, :], onesh[:, :]).then_inc(s_w, 16)
            sp.dma_start(idents[:, :], identh[:, :]).then_inc(s_w, 16)
            sp.dma_start(zbias[:, :], zbh[:, :]).then_inc(s_w, 16)
            if w1_resident and W1_ENG == "sp":
                _w1_dmas(sp)
            for ch in range(NCHUNK):
                sl = ch % 2
                if ch >= 2:
                    sp.wait_ge(s_xd, ch - 1)
                lo, hi = ch * CW, (ch + 1) * CW
                sp.dma_start(x1s[sl][:, 0, :], x1h[0, :, lo:hi]).then_inc(sxs[sl], 16)
                sp.dma_start(x1s[sl][:, 1, :], x1h[1, :, lo:hi]).then_inc(sxs[sl], 16)
                sp.dma_start(x2s[sl][:, 0, :], x2h[0, :, lo:hi]).then_inc(sxs[sl], 16)
                sp.dma_start(x2s[sl][:, 1, :], x2h[1, :, lo:hi]).then_inc(sxs[sl], 16)
            if debug:
                sp.wait_ge(s_e, NGROUP)
                sp.dma_start(gsadbg[:, :, :], gsa[:, :, :]).then_inc(s_o, 16)
                sp.wait_ge(s_r1, 2)
                sp.dma_start(relu1dbg[:, :], relu1[:, :]).then_inc(s_o, 16)
                sp.wait_ge(s_c1, 8)
                sp.dma_start(r1Tdbg[:, :], r1T[:, :]).then_inc(s_o, 16)
                sp.wait_ge(s_r2, 2)
                sp.dma_start(relu2dbg[:, :], relu2[:, :]).then_inc(s_o, 16)
            sp.wait_ge(s_oe, 1)
            sp.dma_start(outh[:, :], outs[:, :]).then_inc(s_o, 16)
            sp.wait_ge(s_o, 16 + (64 if debug else 0))

        def _w1_dmas(eng):
            # 4 big DMAs; transfers overlap the Gram phase
            for s in range(4):
                lo = s * 13 * REP
                hi = min(HW * REP, (s + 1) * 13 * REP)
                eng.dma_start(w1r[:, lo:hi], w1h[:, lo:hi]).then_inc(s_w1a, 16)

        # ---------------- GPSIMD: W1eff load / f32 streaming
        if not w1_resident:

            @block.gpsimd
            def _(gp):
                for s in range(NSLICE):
                    slot = s % 2
                    if s >= 2:
                        gp.wait_ge(s_w1u, s - 1)
                    ncols = SLICES[s] * REP
                    gp.dma_start(
                        w1s[slot][:, 0:ncols],
                        w1h[:, s * SL * REP : s * SL * REP + ncols],
                    ).then_inc(sw1s[slot], 16)
        elif W1_ENG == "gpsimd":

            @block.gpsimd
            def _(gp):
                _w1_dmas(gp)

        # ---------------- PE: all matmuls
        @block.tensor
        def _(pe):
            # Gram phase: G[b]^T[kl, ij] for each local RoI
            for ch in range(NCHUNK):
                sl = ch % 2
                pe.wait_ge(sxs[sl], 64 * (ch // 2 + 1))
                for g in range(GPC):
                    gi = ch * GPC + g
                    q = gi % PSG_RING
                    if gi >= PSG_RING:
                        pe.wait_ge(s_e, gi - PSG_RING + 1)
                        if EVICT_MODE == "split":
                            pe.wait_ge(s_ed, gi - PSG_RING + 1)
                    for bb in range(NG):
                        lb = g * NG + bb
                        for t in range(2):
                            mm = pe.matmul(
                                psG[q][:, bb, :],
                                x2s[sl][:, t, lb * HW : (lb + 1) * HW],
                                x1s[sl][:, t, lb * HW : (lb + 1) * HW],
                                start=(t == 0),
                                stop=(t == 1),
                            )
                    mm.then_inc(s_g, 1)

            # fc1: out1[b,rep] = sum_ij sum_kl G^T[kl, ij*128+b] * W1eff
            pe.wait_ge(s_e, NGROUP)
            if EVICT_MODE == "split":
                pe.wait_ge(s_ed, NGROUP)
            pe.wait_ge(s_w, N_WDMA)
            if w1_resident:
                pe.wait_ge(s_w1a, 64)
                for ij in range(HW):
                    for hf in range(2):
                        pe.matmul(
                            psF[hf][:, :],
                            gsa[:, ij, :],
                            w1r[:, ij * REP + hf * 512 : ij * REP + hf * 512 + 512],
                            start=(ij == 0),
                            stop=False,
                        )
            else:
                for s in range(NSLICE):
                    slot = s % 2
                    pe.wait_ge(sw1s[slot], 16 * (s // 2 + 1))
                    for j in range(SLICES[s]):
                        ij = s * SL + j
                        for hf in range(2):
                            mm = pe.matmul(
                                psF[hf][:, :],
                                gsa[:, ij, :],
                                w1s[slot][
                                    :, j * REP + hf * 512 : j * REP + hf * 512 + 512
                                ],
                                start=(ij == 0),
                                stop=False,
                            )
                    mm.then_inc(s_w1u, 1)
            for hf in range(2):
                pe.matmul(
                    psF[hf][:, :],
                    ones[:, :],
                    b1s[:, hf * 512 : hf * 512 + 512],
                    start=False,
                    stop=True,
                ).then_inc(s_f1, 1)

            # transpose relu1 -> r1T
            pe.wait_ge(s_r1, 2)
            for k in range(8):
                if k >= 2:
                    pe.wait_ge(s_c1, k - 1)
                pe.transpose(
                    psT[k % 2][:, :], relu1[:, k * 128 : (k + 1) * 128], idents[:, :]
                ).then_inc(s_t1, 1)

            # fc2
            for k in range(8):
                pe.wait_ge(s_c1, k + 1)
                for hf in range(2):
                    pe.matmul(
                        psF[hf][:, :],
                        r1T[:, k * 128 : (k + 1) * 128],
                        w2s[:, k * REP + hf * 512 : k * REP + hf * 512 + 512],
                        start=(k == 0),
                        stop=False,
                    )
            for hf in range(2):
                pe.matmul(
                    psF[hf][:, :],
                    ones[:, :],
                    b2s[:, hf * 512 : hf * 512 + 512],
                    start=False,
                    stop=True,
                ).then_inc(s_f2, 1)

            # transpose relu2 -> r2T
            pe.wait_ge(s_r2, 2)
            for k in range(8):
                if k >= 2:
                    pe.wait_ge(s_c2, k - 1)
                pe.transpose(
                    psT[k % 2][:, :], relu2[:, k * 128 : (k + 1) * 128], idents[:, :]
                ).then_inc(s_t2, 1)

            # fc3
            for k in range(8):
                pe.wait_ge(s_c2, k + 1)
                pe.matmul(
                    psO[:, :],
                    r2T[:, k * 128 : (k + 1) * 128],
                    w3s[:, k * 4 : (k + 1) * 4],
                    start=(k == 0),
                    stop=False,
                )
            pe.matmul(
                psO[:, :], ones[:, :], b3s[:, :], start=False, stop=True
            ).then_inc(s_f3, 1)

        # ---------------- ACT: W1 load (own HWDGE ring), PSUM evictions + ReLU
        @block.scalar
        def _(act):
            if w1_resident and W1_ENG == "act":
                _w1_dmas(act)
            for gi in range(NGROUP):
                q = gi % PSG_RING
                act.wait_ge(s_g, gi + 1)
                if EVICT_MODE == "batch":
                    act.activation(
                        gsa[:, :, gi * NG : (gi + 1) * NG],
                        psG[q][:, :, :].rearrange("p b i -> p i b"),
                        mybir.ActivationFunctionType.Copy,
                    ).then_inc(s_e, 1)
                else:
                    bbs = range(2) if EVICT_MODE == "split" else range(NG)
                    for bb in bbs:
                        b = gi * NG + bb
                        last = act.activation(
                            gsa[:, :, b], psG[q][:, bb, :],
                            mybir.ActivationFunctionType.Copy,
                        )
                    last.then_inc(s_e, 1)
                if gi % GPC == GPC - 1:
                    # s_g >= gi+1 proves the chunk's PE matmuls completed, so
                    # its x slot may be overwritten by SP
                    act.nop().then_inc(s_xd, 1)
            act.wait_ge(s_w, N_WDMA)
            for hf in range(2):
                act.wait_ge(s_f1, hf + 1)
                act.activation(
                    relu1[:, hf * 512 : (hf + 1) * 512], psF[hf][:, :],
                    mybir.ActivationFunctionType.Relu, bias=zbias[:, :],
                ).then_inc(s_r1, 1)
            for hf in range(2):
                act.wait_ge(s_f2, hf + 1)
                act.activation(
                    relu2[:, hf * 512 : (hf + 1) * 512], psF[hf][:, :],
                    mybir.ActivationFunctionType.Relu, bias=zbias[:, :],
                ).then_inc(s_r2, 1)
            act.wait_ge(s_f3, 1)
            act.activation(
                outs[:, :], psO[:, :], mybir.ActivationFunctionType.Copy
            ).then_inc(s_oe, 1)

        # ---------------- DVE: split evictions + transpose copybacks
        @block.vector
        def _(dve):
            if EVICT_MODE == "split":
                for gi in range(NGROUP):
                    q = gi % PSG_RING
                    dve.wait_ge(s_g, gi + 1)
                    for bb in range(2, NG):
                        b = gi * NG + bb
                        last = dve.tensor_copy(gsa[:, :, b], psG[q][:, bb, :])
                    last.then_inc(s_ed, 1)
            for k in range(8):
                dve.wait_ge(s_t1, k + 1)
                dve.tensor_copy(
                    r1T[:, k * 128 : (k + 1) * 128], psT[k % 2][:, :]
                ).then_inc(s_c1, 1)
            for k in range(8):
                dve.wait_ge(s_t2, k + 1)
                dve.tensor_copy(
                    r2T[:, k * 128 : (k + 1) * 128], psT[k % 2][:, :]
                ).then_inc(s_c2, 1)

    return nc


def _get_nc(dt):
    key = ("nc", str(dt))
    if key not in _CACHE:
        _CACHE[key] = _build(dt)
    return _CACHE[key]


# ---------------------------------------------------------------- entry
def kernel(patch1, patch2, W1, b1, W2, b2, W3, b3):
    global LAST_EXEC_NS
    dt = BF16 if DT_MODE == "bf16" else F32
    np_dt = np.float32 if dt == F32 else None  # bf16 handled via ml_dtypes

    if dt == BF16:
        import ml_dtypes
        np_dt = ml_dtypes.bfloat16

    patch1 = np.asarray(patch1, dtype=np.float32).reshape(B, C, HW)
    patch2 = np.asarray(patch2, dtype=np.float32).reshape(B, C, HW)
    W1 = np.asarray(W1, dtype=np.float32)
    W2 = np.asarray(W2, dtype=np.float32)
    W3 = np.asarray(W3, dtype=np.float32)
    b1 = np.asarray(b1, dtype=np.float32)
    b2 = np.asarray(b2, dtype=np.float32)
    b3 = np.asarray(b3, dtype=np.float32)

    w1e = _w1eff(W1, np_dt).reshape(HW, HW * REP)
    w2e = np.ascontiguousarray(
        W2.T.reshape(8, 128, REP).transpose(1, 0, 2).reshape(128, 8 * REP)
    ).astype(np_dt)
    w3e = np.ascontiguousarray(
        W3.T.reshape(8, 128, 4).transpose(1, 0, 2).reshape(128, 32)
    ).astype(np_dt)

    shared = {
        "w1h": np.ascontiguousarray(w1e).astype(np_dt, copy=False),
        "w2h": w2e,
        "w3h": w3e,
        "b1h": b1.reshape(1, REP).astype(np_dt),
        "b2h": b2.reshape(1, REP).astype(np_dt),
        "b3h": b3.reshape(1, 4).astype(np_dt),
        "onesh": np.ones((1, 128), dtype=np_dt),
        "identh": np.eye(128, dtype=np.float32).astype(np_dt),
        "zbh": np.zeros((128, 1), dtype=np.float32),
    }

    in_maps = []
    for i in range(N_CORES):
        sl = slice(i * BL, (i + 1) * BL)
        x1 = np.ascontiguousarray(
            patch1[sl].transpose(1, 0, 2).reshape(2, 128, BL * HW)
        ).astype(np_dt)
        x2 = np.ascontiguousarray(
            patch2[sl].transpose(1, 0, 2).reshape(2, 128, BL * HW)
        ).astype(np_dt)
        in_maps.append({"x1h": x1, "x2h": x2, **shared})

    nc = _get_nc(dt)
    trace = os.environ.get("CORR_TRACE", "0") == "1"
    res = run_bass_kernel_spmd(nc, in_maps, list(range(N_CORES)), trace=trace)
    LAST_EXEC_NS = res.exec_time_ns

    out = np.concatenate(
        [res.results[i]["outh"] for i in range(N_CORES)], axis=0
    ).astype(np.float32)
    return out



# revision 14
# speedup vs baseline: 1.0012x; 1.0012x over previous
"""CorrelationHead Trainium2 kernel (parity-class packed).

Math: SpatialCorrelationSampler(patch=16, dil=2) on 7x7 maps -> corr features
are exactly the per-RoI Gram matrix G[b][kl,ij] = sum_c x1[b,c,ij]*x2[b,c,kl],
valid only when (k-i) and (l-j) are both even (parity match). The 49 spatial
positions split into 4 parity classes (i%2,j%2) of sizes 16/12/12/9; valid
(ij,kl) pairs live in the 4 class-diagonal blocks (625 of 2401).

Device layout: spatial columns are host-permuted class-major. Gram per RoI is
computed as 4 class-diagonal PSUM blocks (2 accumulating matmuls over the 256
channels each). Evictions pack blocks into gsaP[row, g, b]: row = class-major
kl (+ row 49 = ones for bias), g = ij index within its class (0..15). Each g
stacks one ij per class on disjoint partition ranges, so fc1 is 16x2 matmuls
of K=50 accumulating into [128b, 1024] PSUM, with b1 folded in via the ones
row. fc2/fc3 via PE transpose + K=128 matmuls as usual.

Sharding: pure data-parallel over the 1024 RoIs -> 128 per each of 8 cores;
weights replicated. PE warm-up dummy matmuls run during the initial input DMA
window to lift the HAM clock throttle before real work.
"""

import os
import numpy as np

import concourse.bass as bass
import concourse.mybir as mybir
from concourse.bass_utils import run_bass_kernel_spmd

# ---------------------------------------------------------------- constants
P = 16
DIL = 2
H = 7
C = 256
B = 1024
REP = 1024
HW = H * H  # 49
N_CORES = 8
BL = B // N_CORES  # 128 RoIs per core

CH = 16                 # RoIs per input chunk
NCHUNK = BL // CH       # 8
NSLOT = 4               # x double-buffer depth
NG = 4                  # RoIs per PSUM gram group
NGROUP = BL // NG       # 32
GPC = CH // NG          # groups per chunk = 4
PSG_RING = 3

NDUMMY = 44             # PE warm-up matmuls (~4.7us cold)

F32 = mybir.dt.float32
BF16 = mybir.dt.bfloat16

LAST_EXEC_NS = None
_CACHE = {}

# ---------------------------------------------------------------- class map
# class order: (even,even), (even,odd), (odd,even), (odd,odd)
_CLS = [(0, 0), (0, 1), (1, 0), (1, 1)]
PERM = []           # class-major list of original ij = i*7+j
OFF = []            # class start offsets within 0..48
SZ = []             # class sizes
for (pi, pj) in _CLS:
    OFF.append(len(PERM))
    n0 = len(PERM)
    for i in range(H):
        if i % 2 != pi:
            continue
        for j in range(H):
            if j % 2 != pj:
                continue
            PERM.append(i * H + j)
    SZ.append(len(PERM) - n0)
assert len(PERM) == HW and SZ == [16, 12, 12, 9]
NGRP = max(SZ)  # 16 fc1 groups
# PSUM/SBUF partition row offsets per class: matmul outputs must start on a
# 32-aligned partition (PE col-group), so class kl-blocks live at 0/32/64/96.
ROF = [0, 32, 64, 96]
BIAS_ROW = 105  # right after the oo block (96..104)
KF = 106        # fc1 contraction rows (class blocks + ones row for bias)


# ---------------------------------------------------------------- host prep
def _w1p(W1, b1, np_dt):
    """[KF, NGRP, REP]: rows ROF[c]+a hold W1[:, feat(ij,kl)] for class-local
    (kl=a, ij=g); row BIAS_ROW group 0 holds b1; all else zero."""
    w = np.zeros((KF, NGRP, REP), dtype=np.float32)
    for c in range(4):
        off, s = OFF[c], SZ[c]
        for a in range(s):
            kl = PERM[off + a]
            k, l = divmod(kl, H)
            for g in range(s):
                ij = PERM[off + g]
                i, j = divmod(ij, H)
                ph = (k - i) // 2 + 7
                pw = (l - j) // 2 + 7
                f = (ph * P + pw) * HW + ij
                w[ROF[c] + a, g, :] = W1[:, f]
    w[BIAS_ROW, 0, :] = b1
    return w.astype(np_dt)


def _ginit(np_dt):
    """gsaP init image: zeros + ones row for bias."""
    g = np.zeros((KF, NGRP, BL), dtype=np.float32)
    g[BIAS_ROW, 0, :] = 1.0
    return g.astype(np_dt)


# ---------------------------------------------------------------- device IR
def _build(dt):
    nc = bass.Bass()

    CW = CH * HW  # 784 columns per chunk

    x1h = nc.dram_tensor("x1h", [2, 128, BL * HW], dt, kind="ExternalInput")
    x2h = nc.dram_tensor("x2h", [2, 128, BL * HW], dt, kind="ExternalInput")
    w1h = nc.dram_tensor("w1h", [KF, NGRP * REP], dt, kind="ExternalInput")
    gih = nc.dram_tensor("gih", [KF, NGRP, BL], dt, kind="ExternalInput")
    w2h = nc.dram_tensor("w2h", [128, 8 * REP], dt, kind="ExternalInput")
    w3h = nc.dram_tensor("w3h", [128, 8 * 4], dt, kind="ExternalInput")
    b2h = nc.dram_tensor("b2h", [1, REP], dt, kind="ExternalInput")
    b3h = nc.dram_tensor("b3h", [1, 4], dt, kind="ExternalInput")
    onesh = nc.dram_tensor("onesh", [1, 128], dt, kind="ExternalInput")
    identh = nc.dram_tensor("identh", [128, 128], dt, kind="ExternalInput")
    zbh = nc.dram_tensor("zbh", [128, 1], F32, kind="ExternalInput")
    dumh = nc.dram_tensor("dumh", [128, 128], dt, kind="ExternalInput")
    outh = nc.dram_tensor("outh", [128, 4], F32, kind="ExternalOutput")

    from contextlib import ExitStack

    with ExitStack() as ctx:
        sb = lambda name, shape, d: ctx.enter_context(nc.sbuf_tensor(name, shape, d))
        ps = lambda name, shape, d: ctx.enter_context(nc.psum_tensor(name, shape, d))
        sem = lambda name: ctx.enter_context(nc.semaphore(name))

        xs = [
            (sb(f"x1s{s}", [128, 2, CW], dt), sb(f"x2s{s}", [128, 2, CW], dt))
            for s in range(NSLOT)
        ]
        gsaP = sb("gsaP", [KF, NGRP, BL], dt)
        w1r = sb("w1r", [KF, NGRP * REP], dt)
        w2s = sb("w2s", [128, 8 * REP], dt)
        w3s = sb("w3s", [128, 8 * 4], dt)
        b2s = sb("b2s", [1, REP], dt)
        b3s = sb("b3s", [1, 4], dt)
        ones = sb("ones", [1, 128], dt)
        idents = sb("idents", [128, 128], dt)
        zbias = sb("zbias", [128, 1], F32)
        dums = sb("dums", [128, 128], dt)
        dscr = sb("dscr", [128, 1], dt)
        relu1 = sb("relu1", [128, REP], dt)
        r1T = sb("r1T", [128, REP], dt)
        relu2 = sb("relu2", [128, REP], dt)
        r2T = sb("r2T", [128, REP], dt)
        outs = sb("outs", [128, 4], F32)

        psG = [ps(f"psG{q}", [ROF[3] + SZ[3], NG, HW], F32) for q in range(PSG_RING)]
        psF = [ps(f"psF{h}", [128, 512], F32) for h in range(2)]
        psT = [ps(f"psT{h}", [128, 128], dt) for h in range(2)]
        psO = ps("psO", [128, 4], F32)

        s_dum = sem("s_dum")
        s_x = [sem(f"s_x{s}") for s in range(NSLOT)]
        s_xd = sem("s_xd")
        s_w = sem("s_w")
        s_w1a = sem("s_w1a")
        s_g = sem("s_g")
        s_e = sem("s_e")
        s_ed = sem("s_ed")
        s_f1 = sem("s_f1")
        s_r1 = sem("s_r1")
        s_t1 = sem("s_t1")
        s_c1 = sem("s_c1")
        s_f2 = sem("s_f2")
        s_r2 = sem("s_r2")
        s_t2 = sem("s_t2")
        s_c2 = sem("s_c2")
        s_f3 = sem("s_f3")
        s_oe = sem("s_oe")
        s_o = sem("s_o")

        block = ctx.enter_context(nc.Block())
        N_WDMA = 7 * 16  # w2,w3 (gpsimd) + b2,b3,ones,idents,zbias (sp)

        # ---------------- SP: dummy init + x input DMAs + final output
        @block.sync
        def _(sp):
            sp.dma_start(dums[:, :], dumh[:, :]).then_inc(s_dum, 16)
            for ch in range(NCHUNK):
                sl = ch % NSLOT
                if ch >= NSLOT:
                    sp.wait_ge(s_xd, ch - NSLOT + 1)
                lo, hi = ch * CW, (ch + 1) * CW
                x1s, x2s = xs[sl]
                sp.dma_start(x1s[:, 0, :], x1h[0, :, lo:hi]).then_inc(s_x[sl], 16)
                sp.dma_start(x1s[:, 1, :], x1h[1, :, lo:hi]).then_inc(s_x[sl], 16)
                sp.dma_start(x2s[:, 0, :], x2h[0, :, lo:hi]).then_inc(s_x[sl], 16)
                sp.dma_start(x2s[:, 1, :], x2h[1, :, lo:hi]).then_inc(s_x[sl], 16)
            sp.dma_start(b2s[:, :], b2h[:, :]).then_inc(s_w, 16)
            sp.dma_start(b3s[:, :], b3h[:, :]).then_inc(s_w, 16)
            sp.dma_start(ones[:, :], onesh[:, :]).then_inc(s_w, 16)
            sp.dma_start(idents[:, :], identh[:, :]).then_inc(s_w, 16)
            sp.dma_start(zbias[:, :], zbh[:, :]).then_inc(s_w, 16)
            sp.wait_ge(s_oe, 1)
            sp.dma_start(outh[:, :], outs[:, :]).then_inc(s_o, 16)
            sp.wait_ge(s_o, 16)

        # ---------------- GPSIMD: fc2/fc3 weights on a separate ring
        @block.gpsimd
        def _(gp):
            gp.dma_start(w2s[:, :], w2h[:, :]).then_inc(s_w, 16)
            gp.dma_start(w3s[:, :], w3h[:, :]).then_inc(s_w, 16)

        # ---------------- PE: warm-up, gram, fc1, transposes, fc2, fc3
        @block.tensor
        def _(pe):
            pe.wait_ge(s_dum, 16)
            for _ in range(NDUMMY):
                pe.matmul(psF[0][:, 0:128], dums[:, :], dums[:, :],
                          start=True, stop=True)

            # Gram phase: 4 class-diagonal blocks per RoI
            for chk in range(NCHUNK):
                sl = chk % NSLOT
                pe.wait_ge(s_x[sl], 64 * (chk // NSLOT + 1))
                for g4 in range(GPC):
                    gi = chk * GPC + g4
                    q = gi % PSG_RING
                    if gi >= PSG_RING:
                        pe.wait_ge(s_e, gi - PSG_RING + 1)
                        pe.wait_ge(s_ed, gi - PSG_RING + 1)
                    for bb in range(NG):
                        lb = g4 * NG + bb
                        base = lb * HW
                        x1s, x2s = xs[sl]
                        # class-outer, channel-half-inner: start=True clears
                        # has_written for the WHOLE bank, so each class's
                        # accumulation pair must finish before the next starts
                        for c in range(4):
                            for t in range(2):
                                o, s, r = OFF[c], SZ[c], ROF[c]
                                mm = pe.matmul(
                                    psG[q][r:r + s, bb, o:o + s],
                                    x2s[:, t, base + o:base + o + s],
                                    x1s[:, t, base + o:base + o + s],
                                    start=(t == 0),
                                    stop=(t == 1),
                                    tile_position=(0, r),
                                )
                    mm.then_inc(s_g, 1)

            # fc1: 16 group matmuls x 2 halves, bias via ones row
            pe.wait_ge(s_e, NGROUP)
            pe.wait_ge(s_ed, NGROUP)
            pe.wait_ge(s_w1a, 80)
            for g in range(NGRP):
                for hf in range(2):
                    mm = pe.matmul(
                        psF[hf][:, :],
                        gsaP[:, g, :],
                        w1r[:, g * REP + hf * 512:g * REP + hf * 512 + 512],
                        start=(g == 0),
                        stop=(g == NGRP - 1),
                    )
                    if g == NGRP - 1:
                        mm.then_inc(s_f1, 1)

            # transpose relu1 -> r1T
            pe.wait_ge(s_w, N_WDMA)
            for k in range(8):
                pe.wait_ge(s_r1, 1 if k < 4 else 2)
                if k >= 2:
                    pe.wait_ge(s_c1, k - 1)
                pe.transpose(
                    psT[k % 2][:, :], relu1[:, k * 128:(k + 1) * 128], idents[:, :]
                ).then_inc(s_t1, 1)

            # fc2
            for k in range(8):
                pe.wait_ge(s_c1, k + 1)
                for hf in range(2):
                    pe.matmul(
                        psF[hf][:, :],
                        r1T[:, k * 128:(k + 1) * 128],
                        w2s[:, k * REP + hf * 512:k * REP + hf * 512 + 512],
                        start=(k == 0),
                        stop=False,
                    )
            for hf in range(2):
                pe.matmul(
                    psF[hf][:, :], ones[:, :], b2s[:, hf * 512:hf * 512 + 512],
                    start=False, stop=True,
                ).then_inc(s_f2, 1)

            # transpose relu2 -> r2T
            for k in range(8):
                pe.wait_ge(s_r2, 1 if k < 4 else 2)
                if k >= 2:
                    pe.wait_ge(s_c2, k - 1)
                pe.transpose(
                    psT[k % 2][:, :], relu2[:, k * 128:(k + 1) * 128], idents[:, :]
                ).then_inc(s_t2, 1)

            # fc3
            for k in range(8):
                pe.wait_ge(s_c2, k + 1)
                pe.matmul(
                    psO[:, :],
                    r2T[:, k * 128:(k + 1) * 128],
                    w3s[:, k * 4:(k + 1) * 4],
                    start=(k == 0),
                    stop=False,
                )
            pe.matmul(
                psO[:, :], ones[:, :], b3s[:, :], start=False, stop=True
            ).then_inc(s_f3, 1)

        # ---------------- ACT: gsaP init + W1 DMAs (own ring), evictions
        # classes 0,3; ReLUs; output eviction
        @block.scalar
        def _(act):
            act.dma_start(gsaP[:, :, :], gih[:, :, :]).then_inc(s_w1a, 16)
            for s4 in range(4):
                lo = s4 * 4 * REP
                hi = (s4 + 1) * 4 * REP
                act.dma_start(w1r[:, lo:hi], w1h[:, lo:hi]).then_inc(s_w1a, 16)
            # warm the activation table while idle
            act.wait_ge(s_dum, 16)
            act.activation(dscr[:, :], dums[:, 0:1],
                           mybir.ActivationFunctionType.Copy)
            act.wait_ge(s_w1a, 16)
            for gi in range(NGROUP):
                q = gi % PSG_RING
                act.wait_ge(s_g, gi + 1)
                for c in (0, 3):
                    o, s, r = OFF[c], SZ[c], ROF[c]
                    last = act.activation(
                        gsaP[r:r + s, 0:s, gi * NG:(gi + 1) * NG],
                        psG[q][r:r + s, :, o:o + s].rearrange("p b i -> p i b"),
                        mybir.ActivationFunctionType.Copy,
                    )
                last.then_inc(s_e, 1)
                if gi % GPC == GPC - 1:
                    act.nop().then_inc(s_xd, 1)
            act.wait_ge(s_w, N_WDMA)
            for hf in range(2):
                act.wait_ge(s_f1, hf + 1)
                act.activation(
                    relu1[:, hf * 512:(hf + 1) * 512], psF[hf][:, :],
                    mybir.ActivationFunctionType.Relu, bias=zbias[:, :],
                ).then_inc(s_r1, 1)
            for hf in range(2):
                act.wait_ge(s_f2, hf + 1)
                act.activation(
                    relu2[:, hf * 512:(hf + 1) * 512], psF[hf][:, :],
                    mybir.ActivationFunctionType.Relu, bias=zbias[:, :],
                ).then_inc(s_r2, 1)
            act.wait_ge(s_f3, 1)
            act.activation(
                outs[:, :], psO[:, :], mybir.ActivationFunctionType.Copy
            ).then_inc(s_oe, 1)

        # ---------------- DVE: evictions classes 1,2 + transpose copybacks
        @block.vector
        def _(dve):
            dve.wait_ge(s_w1a, 16)
            for gi in range(NGROUP):
                q = gi % PSG_RING
                dve.wait_ge(s_g, gi + 1)
                for c in (1, 2):
                    o, s, r = OFF[c], SZ[c], ROF[c]
                    last = dve.tensor_copy(
                        gsaP[r:r + s, 0:s, gi * NG:(gi + 1) * NG],
                        psG[q][r:r + s, :, o:o + s].rearrange("p b i -> p i b"),
                    )
                last.then_inc(s_ed, 1)
            for k in range(8):
                dve.wait_ge(s_t1, k + 1)
                dve.tensor_copy(
                    r1T[:, k * 128:(k + 1) * 128], psT[k % 2][:, :]
                ).then_inc(s_c1, 1)
            for k in range(8):
                dve.wait_ge(s_t2, k + 1)
                dve.tensor_copy(
                    r2T[:, k * 128:(k + 1) * 128], psT[k % 2][:, :]
                ).then_inc(s_c2, 1)

    return nc


def _get_nc(dt):
    key = ("nc", str(dt))
    if key not in _CACHE:
        _CACHE[key] = _build(dt)
    return _CACHE[key]


# ---------------------------------------------------------------- entry
def kernel(patch1, patch2, W1, b1, W2, b2, W3, b3):
    global LAST_EXEC_NS
    import ml_dtypes
    np_dt = ml_dtypes.bfloat16
    dt = BF16

    patch1 = np.asarray(patch1, dtype=np.float32).reshape(B, C, HW)
    patch2 = np.asarray(patch2, dtype=np.float32).reshape(B, C, HW)
    W1 = np.asarray(W1, dtype=np.float32)
    W2 = np.asarray(W2, dtype=np.float32)
    W3 = np.asarray(W3, dtype=np.float32)
    b1 = np.asarray(b1, dtype=np.float32)
    b2 = np.asarray(b2, dtype=np.float32)
    b3 = np.asarray(b3, dtype=np.float32)

    # class-major spatial permutation
    p1 = patch1[:, :, PERM]
    p2 = patch2[:, :, PERM]

    w1p = _w1p(W1, b1, np_dt).reshape(KF, NGRP * REP)
    gini = _ginit(np_dt)
    w2e = np.ascontiguousarray(
        W2.T.reshape(8, 128, REP).transpose(1, 0, 2).reshape(128, 8 * REP)
    ).astype(np_dt)
    w3e = np.ascontiguousarray(
        W3.T.reshape(8, 128, 4).transpose(1, 0, 2).reshape(128, 32)
    ).astype(np_dt)

    shared = {
        "w1h": np.ascontiguousarray(w1p),
        "gih": np.ascontiguousarray(gini),
        "w2h": w2e,
        "w3h": w3e,
        "b2h": b2.reshape(1, REP).astype(np_dt),
        "b3h": b3.reshape(1, 4).astype(np_dt),
        "onesh": np.ones((1, 128), dtype=np_dt),
        "identh": np.eye(128, dtype=np.float32).astype(np_dt),
        "zbh": np.zeros((128, 1), dtype=np.float32),
        "dumh": np.full((128, 128), 0.125, dtype=np_dt),
    }

    in_maps = []
    for i in range(N_CORES):
        sl = slice(i * BL, (i + 1) * BL)
        x1 = np.ascontiguousarray(
            p1[sl].transpose(1, 0, 2).reshape(2, 128, BL * HW)
        ).astype(np_dt)
        x2 = np.ascontiguousarray(
            p2[sl].transpose(1, 0, 2).reshape(2, 128, BL * HW)
        ).astype(np_dt)
        in_maps.append({"x1h": x1, "x2h": x2, **shared})

    nc = _get_nc(dt)
    trace = os.environ.get("CORR_TRACE", "0") == "1"
    res = run_bass_kernel_spmd(nc, in_maps, list(range(N_CORES)), trace=trace)
    LAST_EXEC_NS = res.exec_time_ns

    out = np.concatenate(
        [res.results[i]["outh"] for i in range(N_CORES)], axis=0
    ).astype(np.float32)
    return out
